# revision 15
# baseline (speedup 1.0000x reference)
"""BKT-over-students kernel for Trainium2 (8 NeuronCores, data-parallel over B).

Math: the per-step BKT update
    correct_t = p(1-s) + (1-p)g
    k = p*a_y / (p*a_y + (1-p)*b_y)        a_1=1-s,b_1=g ; a_0=s,b_0=1-g
    p' = clip(k + (1-k)l, eps, 1-eps)
linearises in odds space v = p/(1-p):
    v' = A_t * v + B     with A_t = (a_y/b_y)/(1-l),  B = l/(1-l)
which maps 1:1 onto the DVE tensor_tensor_scan(op0=mult, op1=add)
instruction (one scan per 128 students covers all T steps, fp32 state).
latents[t] is the state BEFORE step t, so the scan's multiplier column t
must be A_{y_{t-1}}: the host ships y right-shifted by one step, and the
scan's initial value init = (v0-B)/A_{y_0} makes column 0 come out as
the prior odds v0 with no separate copy instruction.

Outputs are affine in rr = 1/(1+v):
    latents  = 1 - rr
    corrects = (1-s) + (g-(1-s)) * rr
rr is computed on the Activation engine via func=Reciprocal with bias=1
(raw-emitted InstActivation; the bass helper blocks the func for accuracy
reasons that don't bind at this problem's 2e-2 gate: measured ~1e-5 rel
err for v<=1e10 and an exact 0.0 at v>=1e20 and inf, which matches the
saturating trajectory the reference's upper clip produces).

Division of labor (device time is DMA-roofline-bound at ~14.7us):
 - host: embedding gather + the 64-dim MLP head + per-student scalar
   constants (0.4% of FLOPs; pure per-row param prep per the sharding
   hint) and the f32 upcast of the fp16 outputs.
 - device: all (B, T) work - the y-conditional coefficient A_t, the
   T-step recurrence, the reciprocal map and both output tensors,
   streamed out as fp16 (adds ~2.5e-4 norm err vs the 2e-2 gate).

Per 128-student chunk (8 per core), engines balanced ~12.5-12.9us each:
    at  = A0 + dA*y       Act / Pool on chunks {1,3,5,6}   int8 -> f32
    v   = scan(at, B, v0') DVE                             f32
    rr  = 1/(1+v)          Act Reciprocal                  f32 -> fp16
    lat = 1 - rr           DVE fp16 (4x mode, 327ns)
    cor = oms + dsg*rr     DVE / Pool on chunks {1,2,4,5}

Schedule: chunk 0 runs at (P,512) half-granularity with per-half stores
so the output stream starts ~3us earlier; dummy activations at t=0 hoist
the two LoadActFuncSet (2.6us) into the initial DMA latency; cst loads
on the Act queue in parallel with y chunk 0 on SP. Stores pair chunks
(2,3) and (4,5) into 4KB DRAM runs; 0,1,6,7 store singly (2KB runs, same
DMA rate) to shorten fill and drain. Layout: student d = 8p + c so every
y load and store sees contiguous runs per partition.
"""

import numpy as np

import concourse.bacc as bacc
import concourse.tile as tile
from concourse import mybir
from concourse.bass_utils import run_bass_kernel_spmd

NCORES = 8
B, T = 8192, 1024
BC = B // NCORES          # students per core
P = 128
NCHUNK = BC // P          # 128-student chunks per core
NK = 6                    # packed per-student constants
EPS = 1e-6
F32 = mybir.dt.float32
F16 = mybir.dt.float16
I8 = mybir.dt.int8
ALU = mybir.AluOpType
ACTF = mybir.ActivationFunctionType

# cst column offsets (per chunk): dA, A0, B, init, dsg, oms
K_DA, K_A0, K_B, K_INIT, K_DSG, K_OMS = range(NK)
AT_POOL = (1, 3, 5, 6)    # chunks whose `at` runs on Pool
CH_POOL = (1, 2, 4, 5)    # chunks whose `cor` runs on Pool
PS_ACT = ()               # chunks whose `lat` runs on Act (rest: DVE)


def _act_recip(nc, out_ap, in_ap):
    """out = 1/(in + 1) on the Activation engine (raw InstActivation;
    the bass helper refuses func=Reciprocal)."""
    nc.scalar.add_instruction(
        mybir.InstActivation(
            name=nc.scalar.bass.get_next_instruction_name(),
            func=ACTF.Reciprocal,
            ins=[
                nc.scalar.lower_ap(in_ap),
                mybir.ImmediateValue(dtype=F32, value=1.0),  # bias
                mybir.ImmediateValue(dtype=F32, value=1.0),  # scale
                mybir.ImmediateValue(dtype=F32, value=0.0),  # alpha
            ],
            outs=[nc.scalar.lower_ap(out_ap)],
        )
    )


def _build_bass():
    nc = bacc.Bacc("TRN2", target_bir_lowering=False, debug=False, num_devices=NCORES)

    y = nc.declare_dram_parameter("y", [BC, T], I8, isOutput=False)
    cst = nc.declare_dram_parameter("cst", [P, NCHUNK * NK], F32, isOutput=False)
    corrects = nc.declare_dram_parameter("corrects", [BC, T], F16, isOutput=True)
    latents = nc.declare_dram_parameter("latents", [BC, T], F16, isOutput=True)
    # DRAM row r = student d = 8*p + c  (partition p, chunk c)
    y3 = y.rearrange("(p c) t -> p c t", p=P, c=NCHUNK)
    lat3 = latents.rearrange("(p c) t -> p c t", p=P, c=NCHUNK)
    cor3 = corrects.rearrange("(p c) t -> p c t", p=P, c=NCHUNK)

    with tile.TileContext(nc) as tc:
        with (
            tc.tile_pool(name="singles", bufs=1) as singles,
            tc.tile_pool(name="work", bufs=4) as work,
            tc.tile_pool(name="pair", bufs=2) as pair,
        ):
            # dummy activations on a memset tile: no DMA dependency, so the
            # LoadActFuncSet each triggers runs during the input DMA latency
            # instead of on the critical path. The Reciprocal warmup is
            # emitted after at(0a) below: its table load then overlaps the
            # y-gated wait without delaying at(0a) on the in-order engine.
            warm = singles.tile([P, 1], F32)
            nc.gpsimd.memset(warm[:], 1.0)
            wout = singles.tile([P, 2], F16)
            nc.scalar.activation(out=wout[:, 0:1], in_=warm[:], func=ACTF.Relu)

            # cst first on SP, y chunk 0 on the Act queue: the two configs
            # run on different sequencers and HWDGE serves cst's 24KB before
            # y0, so at(0a)'s inputs are both ready ~3.3us in.
            cstb = singles.tile([P, NCHUNK * NK], F32)
            nc.sync.dma_start(out=cstb[:], in_=cst[:])
            yt = singles.tile([P, NCHUNK * T], I8)
            ytv = yt[:].rearrange("p (c t) -> p c t", c=NCHUNK)
            nc.scalar.dma_start(out=ytv[:, 0:1, :], in_=y3[:, 0:1, :])
            nc.sync.dma_start(out=ytv[:, 1:2, :], in_=y3[:, 1:2, :])
            nc.sync.dma_start(out=ytv[:, 2:4, :], in_=y3[:, 2:4, :])
            nc.sync.dma_start(out=ytv[:, 4:8, :], in_=y3[:, 4:8, :])

            def col(c, k):
                i = c * NK + k
                return cstb[:, i : i + 1]

            def chain(c, lo, hi, ll_init, at_eng, ps_dst, ch_dst, post_at=None):
                """at/scan/rr/lat/cor for chunk c columns [lo, hi)."""
                n = hi - lo
                ysl = yt[:, c * T + lo : c * T + hi]
                at = work.tile([P, n], F32, tag=f"at{n}")
                if at_eng is nc.scalar:
                    # A_t > 0, so Relu is a no-op (int8 in, f32 out)
                    nc.scalar.activation(
                        out=at[:], in_=ysl, func=ACTF.Relu,
                        scale=col(c, K_DA), bias=col(c, K_A0),
                    )
                else:
                    at_eng.tensor_scalar(
                        out=at[:], in0=ysl, scalar1=col(c, K_DA),
                        scalar2=col(c, K_A0), op0=ALU.mult, op1=ALU.add,
                    )
                if post_at is not None:
                    post_at()
                ll = work.tile([P, n], F32, tag=f"ll{n}")
                nc.vector.tensor_tensor_scan(
                    out=ll[:], data0=at[:],
                    data1=col(c, K_B).to_broadcast([P, n]),
                    initial=ll_init, op0=ALU.mult, op1=ALU.add,
                )
                rr = work.tile([P, n], F16, tag=f"rr{n}")
                _act_recip(nc, rr[:], ll[:])
                if c in PS_ACT:
                    # lat = relu(1 - rr); rr <= 1 so Relu is a no-op. Act has
                    # slack and Relu's table is already loaded.
                    nc.scalar.activation(
                        out=ps_dst, in_=rr[:], func=ACTF.Relu,
                        scale=-1.0, bias=1.0,
                    )
                else:
                    nc.vector.tensor_scalar(
                        out=ps_dst, in0=rr[:], scalar1=-1.0, scalar2=1.0,
                        op0=ALU.mult, op1=ALU.add,
                    )
                ch_eng = nc.gpsimd if c in CH_POOL else nc.vector
                ch_eng.tensor_scalar(
                    out=ch_dst, in0=rr[:], scalar1=col(c, K_DSG),
                    scalar2=col(c, K_OMS), op0=ALU.mult, op1=ALU.add,
                )
                return ll

            def store1(c, lo, hi, ps_src, ch_src):
                """Single-chunk store of columns [lo, hi) (SP queue only:
                DMA configs on the Act queue block rr dispatch in-order)."""
                nc.sync.dma_start(
                    out=lat3[:, c : c + 1, lo:hi],
                    in_=ps_src.rearrange("p (c t) -> p c t", c=1))
                nc.sync.dma_start(
                    out=cor3[:, c : c + 1, lo:hi],
                    in_=ch_src.rearrange("p (c t) -> p c t", c=1))

            # ---- chunk 0: half-granularity, per-half stores ----
            H = T // 2
            ps0 = work.tile([P, T], F16, tag="ps0")
            ch0 = work.tile([P, T], F16, tag="ch0")
            warm_recip = lambda: _act_recip(nc, wout[:, 1:2], warm[:])
            ll0 = chain(0, 0, H, col(0, K_INIT), nc.scalar,
                        ps0[:, 0:H], ch0[:, 0:H], post_at=warm_recip)
            store1(0, 0, H, ps0[:, 0:H], ch0[:, 0:H])
            chain(0, H, T, ll0[:, H - 1 : H], nc.scalar,
                  ps0[:, H:T], ch0[:, H:T])
            store1(0, H, T, ps0[:, H:T], ch0[:, H:T])

            # ---- chunk 1: full, single stores ----
            ps1 = work.tile([P, T], F16, tag="ps0")
            ch1 = work.tile([P, T], F16, tag="ch0")
            chain(1, 0, T, col(1, K_INIT), nc.gpsimd, ps1[:], ch1[:])
            store1(1, 0, T, ps1[:], ch1[:])

            # ---- chunks (2,3) and (4,5): pair-granularity rr/lat + stores.
            # rr and lat have no per-chunk scalars, so one (P, 2T) pass
            # covers both chunks and saves an init overhead per pair on the
            # binding Activation engine.
            for g in (1, 2):
                llp = pair.tile([P, 2 * T], F32, tag="llp")
                rrp = pair.tile([P, 2 * T], F16, tag="rrp")
                ps2 = pair.tile([P, 2 * T], F16, tag="ps")
                ch2 = pair.tile([P, 2 * T], F16, tag="ch")
                for half in range(2):
                    c = 2 * g + half
                    hsl = slice(half * T, (half + 1) * T)
                    ysl = yt[:, c * T : (c + 1) * T]
                    at = work.tile([P, T], F32, tag="at1024")
                    if c in AT_POOL:
                        nc.gpsimd.tensor_scalar(
                            out=at[:], in0=ysl, scalar1=col(c, K_DA),
                            scalar2=col(c, K_A0), op0=ALU.mult, op1=ALU.add,
                        )
                    else:
                        nc.scalar.activation(
                            out=at[:], in_=ysl, func=ACTF.Relu,
                            scale=col(c, K_DA), bias=col(c, K_A0),
                        )
                    nc.vector.tensor_tensor_scan(
                        out=llp[:, hsl], data0=at[:],
                        data1=col(c, K_B).to_broadcast([P, T]),
                        initial=col(c, K_INIT), op0=ALU.mult, op1=ALU.add,
                    )
                _act_recip(nc, rrp[:], llp[:])
                nc.vector.tensor_scalar(
                    out=ps2[:], in0=rrp[:], scalar1=-1.0, scalar2=1.0,
                    op0=ALU.mult, op1=ALU.add,
                )
                for half in range(2):
                    c = 2 * g + half
                    hsl = slice(half * T, (half + 1) * T)
                    ch_eng = nc.gpsimd if c in CH_POOL else nc.vector
                    ch_eng.tensor_scalar(
                        out=ch2[:, hsl], in0=rrp[:, hsl], scalar1=col(c, K_DSG),
                        scalar2=col(c, K_OMS), op0=ALU.mult, op1=ALU.add,
                    )
                sl = slice(2 * g, 2 * g + 2)
                nc.sync.dma_start(
                    out=lat3[:, sl, :], in_=ps2[:].rearrange("p (c t) -> p c t", c=2)
                )
                nc.sync.dma_start(
                    out=cor3[:, sl, :], in_=ch2[:].rearrange("p (c t) -> p c t", c=2)
                )

            # ---- chunk 6: full, single stores ----
            ps6 = work.tile([P, T], F16, tag="ps0")
            ch6 = work.tile([P, T], F16, tag="ch0")
            chain(6, 0, T, col(6, K_INIT), nc.gpsimd, ps6[:], ch6[:])
            store1(6, 0, T, ps6[:], ch6[:])

            # ---- chunk 7: half-granularity (short drain) ----
            ps7 = work.tile([P, T], F16, tag="ps0")
            ch7 = work.tile([P, T], F16, tag="ch0")
            ll7 = chain(7, 0, H, col(7, K_INIT), nc.scalar,
                        ps7[:, 0:H], ch7[:, 0:H])
            store1(7, 0, H, ps7[:, 0:H], ch7[:, 0:H])
            chain(7, H, T, ll7[:, H - 1 : H], nc.scalar,
                  ps7[:, H:T], ch7[:, H:T])
            store1(7, H, T, ps7[:, H:T], ch7[:, H:T])
    nc.compile()
    return nc


_NC_CACHE = None


def _get_nc():
    global _NC_CACHE
    if _NC_CACHE is None:
        _NC_CACHE = _build_bass()
    return _NC_CACHE


def kernel(X, y, embed, W0, b0, W1, b1, Wout, bout):
    X = np.asarray(X).astype(np.int64)
    y8 = np.asarray(y, dtype=np.int8)
    embed = np.asarray(embed, dtype=np.float32)
    W0 = np.asarray(W0, dtype=np.float32)
    W1 = np.asarray(W1, dtype=np.float32)
    Wout = np.asarray(Wout, dtype=np.float32)
    b0 = np.asarray(b0, dtype=np.float32).reshape(-1)
    b1 = np.asarray(b1, dtype=np.float32).reshape(-1)
    bout_v = np.asarray(bout, dtype=np.float32).reshape(-1)

    # per-student params: gather + 64-dim MLP head (f32, mirrors reference)
    h = embed[X]
    h = np.maximum(h @ W0 + b0, 0.0).astype(np.float32)
    h = np.maximum(h @ W1 + b1, 0.0).astype(np.float32)
    z = (h @ Wout + bout_v).astype(np.float32)
    params = np.clip(1.0 / (1.0 + np.exp(-z, dtype=np.float32)), EPS, 1.0 - EPS)

    pd = params.astype(np.float64)
    l, g, s, prior = pd[:, 0], pd[:, 1], pd[:, 2], pd[:, 3]
    A1 = (1.0 - s) / (g * (1.0 - l))
    A0 = s / ((1.0 - g) * (1.0 - l))
    Bv = l / (1.0 - l)
    v0 = prior / (1.0 - prior)
    Ay0 = np.where(y8[:, 0] > 0, A1, A0)
    init = (v0 - Bv) / Ay0
    dsg = g - (1.0 - s)
    oms = 1.0 - s
    consts = np.stack(
        [A1 - A0, A0, Bv, init, dsg, oms], axis=-1
    ).astype(np.float32)                                   # (B, NK)

    # latents[t] needs A_{y_{t-1}}: ship y right-shifted one step. Column 0
    # repeats y_0, consistent with init = (v0-B)/A_{y_0} producing v0 there.
    ysh = np.concatenate([y8[:, :1], y8[:, :-1]], axis=1)

    nc = _get_nc()
    in_maps = []
    for c in range(NCORES):
        rows = slice(c * BC, (c + 1) * BC)
        # student d = 8p + c -> cst[p, c*NK + k]
        cstm = np.ascontiguousarray(consts[rows].reshape(P, NCHUNK * NK))
        in_maps.append({
            "y": np.ascontiguousarray(ysh[rows]),
            "cst": cstm,
        })
    res = run_bass_kernel_spmd(nc, in_maps, list(range(NCORES)))
    corrects = np.concatenate(
        [res.results[c]["corrects"] for c in range(NCORES)], axis=0
    ).astype(np.float32)
    latents = np.concatenate(
        [res.results[c]["latents"] for c in range(NCORES)], axis=0
    ).astype(np.float32)
    return corrects, latents


# revision 17
# speedup vs baseline: 1.0777x; 1.0777x over previous
"""BKT-over-students kernel for Trainium2 (8 NeuronCores, data-parallel over B).

Math: the per-step BKT update
    correct_t = p(1-s) + (1-p)g
    k = p*a_y / (p*a_y + (1-p)*b_y)        a_1=1-s,b_1=g ; a_0=s,b_0=1-g
    p' = clip(k + (1-k)l, eps, 1-eps)
linearises in odds space v = p/(1-p):
    v' = A_t * v + B     with A_t = (a_y/b_y)/(1-l),  B = l/(1-l)
which maps 1:1 onto the DVE tensor_tensor_scan(op0=mult, op1=add)
instruction (one scan per 128 students covers all T steps, fp32 state).
latents[t] is the state BEFORE step t, so the scan's multiplier column t
must be A_{y_{t-1}}: the host ships y right-shifted by one step, and the
scan's initial value init = (v0-B)/A_{y_0} makes column 0 come out as
the prior odds v0 with no separate copy instruction.

Outputs are affine in rr = 1/(1+v):
    latents  = 1 - rr
    corrects = (1-s) + (g-(1-s)) * rr
rr is computed on the Activation engine via func=Reciprocal with bias=1
(raw-emitted InstActivation; the bass helper blocks the func for accuracy
reasons that don't bind at this problem's 2e-2 gate: measured ~1e-5 rel
err for v<=1e10 and an exact 0.0 at v>=1e20 and inf, which matches the
saturating trajectory the reference's upper clip produces).

Division of labor (device time is DMA-roofline-bound at ~14.7us):
 - host: embedding gather + the 64-dim MLP head + per-student scalar
   constants (0.4% of FLOPs; pure per-row param prep per the sharding
   hint) and the f32 upcast of the fp16 outputs.
 - device: all (B, T) work - the y-conditional coefficient A_t, the
   T-step recurrence, the reciprocal map and both output tensors,
   streamed out as fp16 (adds ~2.5e-4 norm err vs the 2e-2 gate).

Per 128-student chunk (8 per core), engines balanced ~12.5-12.9us each:
    at  = A0 + dA*y       Act / Pool on chunks {1,3,5,6}   int8 -> f32
    v   = scan(at, B, v0') DVE                             f32
    rr  = 1/(1+v)          Act Reciprocal                  f32 -> fp16
    lat = 1 - rr           DVE fp16 (4x mode, 327ns)
    cor = oms + dsg*rr     DVE / Pool on chunks {1,2,4,5}

Schedule: chunk 0 runs at (P,512) half-granularity with per-half stores
so the output stream starts ~3us earlier; dummy activations at t=0 hoist
the two LoadActFuncSet (2.6us) into the initial DMA latency; cst loads
on the Act queue in parallel with y chunk 0 on SP. Stores pair chunks
(2,3) and (4,5) into 4KB DRAM runs; 0,1,6,7 store singly (2KB runs, same
DMA rate) to shorten fill and drain. Layout: student d = 8p + c so every
y load and store sees contiguous runs per partition.
"""

import numpy as np

import concourse.bacc as bacc
import concourse.tile as tile
from concourse import mybir
from concourse.bass_utils import run_bass_kernel_spmd

NCORES = 8
B, T = 8192, 1024
BC = B // NCORES          # students per core
P = 128
NCHUNK = BC // P          # 128-student chunks per core
NK = 6                    # packed per-student constants
EPS = 1e-6
F32 = mybir.dt.float32
F16 = mybir.dt.float16
I8 = mybir.dt.int8
ALU = mybir.AluOpType
ACTF = mybir.ActivationFunctionType

# cst column offsets (per chunk): dA, A0, B, init, dsg, oms
K_DA, K_A0, K_B, K_INIT, K_DSG, K_OMS = range(NK)
AT_POOL = (1, 3, 5, 6)    # chunks whose `at` runs on Pool
CH_POOL = (1, 2, 4, 5)    # chunks whose `cor` runs on Pool
PS_ACT = ()               # chunks whose `lat` runs on Act (rest: DVE)


def _act_recip(nc, out_ap, in_ap):
    """out = 1/(in + 1) on the Activation engine (raw InstActivation;
    the bass helper refuses func=Reciprocal)."""
    nc.scalar.add_instruction(
        mybir.InstActivation(
            name=nc.scalar.bass.get_next_instruction_name(),
            func=ACTF.Reciprocal,
            ins=[
                nc.scalar.lower_ap(in_ap),
                mybir.ImmediateValue(dtype=F32, value=1.0),  # bias
                mybir.ImmediateValue(dtype=F32, value=1.0),  # scale
                mybir.ImmediateValue(dtype=F32, value=0.0),  # alpha
            ],
            outs=[nc.scalar.lower_ap(out_ap)],
        )
    )


def _build_bass():
    nc = bacc.Bacc("TRN2", target_bir_lowering=False, debug=False, num_devices=NCORES)

    y = nc.declare_dram_parameter("y", [BC, T], I8, isOutput=False)
    cst = nc.declare_dram_parameter("cst", [P, NCHUNK * NK], F32, isOutput=False)
    corrects = nc.declare_dram_parameter("corrects", [BC, T], F16, isOutput=True)
    latents = nc.declare_dram_parameter("latents", [BC, T], F16, isOutput=True)
    # DRAM row r = student d = 8*p + c  (partition p, chunk c)
    y3 = y.rearrange("(p c) t -> p c t", p=P, c=NCHUNK)
    lat3 = latents.rearrange("(p c) t -> p c t", p=P, c=NCHUNK)
    cor3 = corrects.rearrange("(p c) t -> p c t", p=P, c=NCHUNK)

    with tile.TileContext(nc) as tc:
        with (
            tc.tile_pool(name="singles", bufs=1) as singles,
            tc.tile_pool(name="work", bufs=4) as work,
            tc.tile_pool(name="pair", bufs=2) as pair,
        ):
            # dummy activations on a memset tile: no DMA dependency, so the
            # LoadActFuncSet each triggers runs during the input DMA latency
            # instead of on the critical path. The Reciprocal warmup is
            # emitted after at(0a) below: its table load then overlaps the
            # y-gated wait without delaying at(0a) on the in-order engine.
            warm = singles.tile([P, 1], F32)
            nc.gpsimd.memset(warm[:], 1.0)
            wout = singles.tile([P, 2], F16)
            nc.scalar.activation(out=wout[:, 0:1], in_=warm[:], func=ACTF.Relu)

            # y chunk 0 first on SP; cst on the Act queue (its config runs in
            # parallel); remaining y in single/paired loads on SP.
            yt = singles.tile([P, NCHUNK * T], I8)
            ytv = yt[:].rearrange("p (c t) -> p c t", c=NCHUNK)
            nc.sync.dma_start(out=ytv[:, 0:1, :], in_=y3[:, 0:1, :])
            cstb = singles.tile([P, NCHUNK * NK], F32)
            nc.scalar.dma_start(out=cstb[:], in_=cst[:])
            nc.sync.dma_start(out=ytv[:, 1:2, :], in_=y3[:, 1:2, :])
            nc.sync.dma_start(out=ytv[:, 2:4, :], in_=y3[:, 2:4, :])
            nc.sync.dma_start(out=ytv[:, 4:6, :], in_=y3[:, 4:6, :])
            nc.sync.dma_start(out=ytv[:, 6:8, :], in_=y3[:, 6:8, :])

            def col(c, k):
                i = c * NK + k
                return cstb[:, i : i + 1]

            def chain(c, lo, hi, ll_init, at_eng, ps_dst, ch_dst, post_at=None):
                """at/scan/rr/lat/cor for chunk c columns [lo, hi)."""
                n = hi - lo
                ysl = yt[:, c * T + lo : c * T + hi]
                at = work.tile([P, n], F32, tag=f"at{n}")
                if at_eng is nc.scalar:
                    # A_t > 0, so Relu is a no-op (int8 in, f32 out)
                    nc.scalar.activation(
                        out=at[:], in_=ysl, func=ACTF.Relu,
                        scale=col(c, K_DA), bias=col(c, K_A0),
                    )
                else:
                    at_eng.tensor_scalar(
                        out=at[:], in0=ysl, scalar1=col(c, K_DA),
                        scalar2=col(c, K_A0), op0=ALU.mult, op1=ALU.add,
                    )
                if post_at is not None:
                    post_at()
                ll = work.tile([P, n], F32, tag=f"ll{n}")
                nc.vector.tensor_tensor_scan(
                    out=ll[:], data0=at[:],
                    data1=col(c, K_B).to_broadcast([P, n]),
                    initial=ll_init, op0=ALU.mult, op1=ALU.add,
                )
                rr = work.tile([P, n], F16, tag=f"rr{n}")
                _act_recip(nc, rr[:], ll[:])
                if c in PS_ACT:
                    # lat = relu(1 - rr); rr <= 1 so Relu is a no-op. Act has
                    # slack and Relu's table is already loaded.
                    nc.scalar.activation(
                        out=ps_dst, in_=rr[:], func=ACTF.Relu,
                        scale=-1.0, bias=1.0,
                    )
                else:
                    nc.vector.tensor_scalar(
                        out=ps_dst, in0=rr[:], scalar1=-1.0, scalar2=1.0,
                        op0=ALU.mult, op1=ALU.add,
                    )
                ch_eng = nc.gpsimd if c in CH_POOL else nc.vector
                ch_eng.tensor_scalar(
                    out=ch_dst, in0=rr[:], scalar1=col(c, K_DSG),
                    scalar2=col(c, K_OMS), op0=ALU.mult, op1=ALU.add,
                )
                return ll

            def store1(c, lo, hi, ps_src, ch_src):
                """Single-chunk store of columns [lo, hi) (SP queue only:
                DMA configs on the Act queue block rr dispatch in-order)."""
                nc.sync.dma_start(
                    out=lat3[:, c : c + 1, lo:hi],
                    in_=ps_src.rearrange("p (c t) -> p c t", c=1))
                nc.sync.dma_start(
                    out=cor3[:, c : c + 1, lo:hi],
                    in_=ch_src.rearrange("p (c t) -> p c t", c=1))

            # ---- chunk 0: half-granularity, per-half stores ----
            H = T // 2
            ps0 = work.tile([P, T], F16, tag="ps0")
            ch0 = work.tile([P, T], F16, tag="ch0")
            warm_recip = lambda: _act_recip(nc, wout[:, 1:2], warm[:])
            ll0 = chain(0, 0, H, col(0, K_INIT), nc.scalar,
                        ps0[:, 0:H], ch0[:, 0:H], post_at=warm_recip)
            store1(0, 0, H, ps0[:, 0:H], ch0[:, 0:H])
            chain(0, H, T, ll0[:, H - 1 : H], nc.scalar,
                  ps0[:, H:T], ch0[:, H:T])
            store1(0, H, T, ps0[:, H:T], ch0[:, H:T])

            # ---- chunk 1: full, single stores ----
            ps1 = work.tile([P, T], F16, tag="ps0")
            ch1 = work.tile([P, T], F16, tag="ch0")
            chain(1, 0, T, col(1, K_INIT), nc.gpsimd, ps1[:], ch1[:])
            store1(1, 0, T, ps1[:], ch1[:])

            # ---- chunks (2,3) and (4,5): paired stores (4KB DRAM runs) ----
            for g in (1, 2):
                ps2 = pair.tile([P, 2 * T], F16, tag="ps")
                ch2 = pair.tile([P, 2 * T], F16, tag="ch")
                for half in range(2):
                    c = 2 * g + half
                    hsl = slice(half * T, (half + 1) * T)
                    at_eng = nc.gpsimd if c in AT_POOL else nc.scalar
                    chain(c, 0, T, col(c, K_INIT), at_eng,
                          ps2[:, hsl], ch2[:, hsl])
                sl = slice(2 * g, 2 * g + 2)
                nc.sync.dma_start(
                    out=lat3[:, sl, :], in_=ps2[:].rearrange("p (c t) -> p c t", c=2)
                )
                nc.sync.dma_start(
                    out=cor3[:, sl, :], in_=ch2[:].rearrange("p (c t) -> p c t", c=2)
                )

            # ---- chunk 6: full, single stores ----
            ps6 = work.tile([P, T], F16, tag="ps0")
            ch6 = work.tile([P, T], F16, tag="ch0")
            chain(6, 0, T, col(6, K_INIT), nc.gpsimd, ps6[:], ch6[:])
            store1(6, 0, T, ps6[:], ch6[:])

            # ---- chunk 7: half-granularity (short drain) ----
            ps7 = work.tile([P, T], F16, tag="ps0")
            ch7 = work.tile([P, T], F16, tag="ch0")
            ll7 = chain(7, 0, H, col(7, K_INIT), nc.scalar,
                        ps7[:, 0:H], ch7[:, 0:H])
            store1(7, 0, H, ps7[:, 0:H], ch7[:, 0:H])
            chain(7, H, T, ll7[:, H - 1 : H], nc.scalar,
                  ps7[:, H:T], ch7[:, H:T])
            store1(7, H, T, ps7[:, H:T], ch7[:, H:T])
    nc.compile()
    return nc


_NC_CACHE = None


def _get_nc():
    global _NC_CACHE
    if _NC_CACHE is None:
        _NC_CACHE = _build_bass()
    return _NC_CACHE


def kernel(X, y, embed, W0, b0, W1, b1, Wout, bout):
    X = np.asarray(X).astype(np.int64)
    y8 = np.asarray(y, dtype=np.int8)
    embed = np.asarray(embed, dtype=np.float32)
    W0 = np.asarray(W0, dtype=np.float32)
    W1 = np.asarray(W1, dtype=np.float32)
    Wout = np.asarray(Wout, dtype=np.float32)
    b0 = np.asarray(b0, dtype=np.float32).reshape(-1)
    b1 = np.asarray(b1, dtype=np.float32).reshape(-1)
    bout_v = np.asarray(bout, dtype=np.float32).reshape(-1)

    # per-student params: gather + 64-dim MLP head (f32, mirrors reference)
    h = embed[X]
    h = np.maximum(h @ W0 + b0, 0.0).astype(np.float32)
    h = np.maximum(h @ W1 + b1, 0.0).astype(np.float32)
    z = (h @ Wout + bout_v).astype(np.float32)
    params = np.clip(1.0 / (1.0 + np.exp(-z, dtype=np.float32)), EPS, 1.0 - EPS)

    pd = params.astype(np.float64)
    l, g, s, prior = pd[:, 0], pd[:, 1], pd[:, 2], pd[:, 3]
    A1 = (1.0 - s) / (g * (1.0 - l))
    A0 = s / ((1.0 - g) * (1.0 - l))
    Bv = l / (1.0 - l)
    v0 = prior / (1.0 - prior)
    Ay0 = np.where(y8[:, 0] > 0, A1, A0)
    init = (v0 - Bv) / Ay0
    dsg = g - (1.0 - s)
    oms = 1.0 - s
    consts = np.stack(
        [A1 - A0, A0, Bv, init, dsg, oms], axis=-1
    ).astype(np.float32)                                   # (B, NK)

    # latents[t] needs A_{y_{t-1}}: ship y right-shifted one step. Column 0
    # repeats y_0, consistent with init = (v0-B)/A_{y_0} producing v0 there.
    ysh = np.concatenate([y8[:, :1], y8[:, :-1]], axis=1)

    nc = _get_nc()
    in_maps = []
    for c in range(NCORES):
        rows = slice(c * BC, (c + 1) * BC)
        # student d = 8p + c -> cst[p, c*NK + k]
        cstm = np.ascontiguousarray(consts[rows].reshape(P, NCHUNK * NK))
        in_maps.append({
            "y": np.ascontiguousarray(ysh[rows]),
            "cst": cstm,
        })
    res = run_bass_kernel_spmd(nc, in_maps, list(range(NCORES)))
    corrects = np.concatenate(
        [res.results[c]["corrects"] for c in range(NCORES)], axis=0
    ).astype(np.float32)
    latents = np.concatenate(
        [res.results[c]["latents"] for c in range(NCORES)], axis=0
    ).astype(np.float32)
    return corrects, latents
